# revision 1
# baseline (speedup 1.0000x reference)
"""CascadeMatching Trainium2 kernel (8-core SPMD, Bass/Tile).

Sharding: 8 cores = (2 batches x 2 directions x 2 query-halves). Each core:
queries [9600,64], full key table [19200,64], idx [9600,100].

Device algorithm per core (matches jax reference within fp rounding):
  aug[j] = 0.125*fk[j] * (mk[j] ? 1 : 2^64)        (masked keys -> huge rows)
  gather aug rows by idx via SWDGE dma_gather (4 queues, 1024 idx/inst)
  s' = <0.125*fq[l], gathered>                      (DVE mul + segmented reduce)
  sim = |s'| > 1e8 ? -1e9 : s'                      (magnitude mask detect)
  sim = sim*mq + (mq-1)*1e9                         (query mask)
  conf = softmax(10*sim); k* = first-occurrence argmax; nconf = max conf
Host: next_idx = idx[l, k*]; host also pre-wraps gather indices (int16,
k-major, wrap-16, group-replicated) as part of sharding.
"""

import numpy as np

import concourse.bass as bass
import concourse.bacc as bacc
import concourse.tile as tile
from concourse import mybir
from concourse.bass_utils import run_bass_kernel_spmd

F32 = mybir.dt.float32
I32 = mybir.dt.int32
I16 = mybir.dt.int16
U32 = mybir.dt.uint32

P = 128
C = 64
K = 100
L = 9600       # queries per core (half of HW0)
V = 19200
TWO64 = float(2.0**64)
THRESH = 1.0e8
NEG = -1.0e9
GQ = 8         # k-slots per gather instruction (1024 indices)
NQUEUE = 4


def _bcast_mid(ap, n_mid):
    return bass.AP(tensor=ap.tensor, offset=ap.offset,
                   ap=[ap.ap[0], [0, n_mid], ap.ap[1]])


def _bcast_last(ap, n):
    return bass.AP(tensor=ap.tensor, offset=ap.offset, ap=[ap.ap[0], [0, n]])


def _build(nc):
    NCH = L // P
    RT = V // P
    W = P * K // 16

    fq = nc.declare_dram_parameter("fq", [L, C], F32, isOutput=False)
    fk = nc.declare_dram_parameter("fk", [V, C], F32, isOutput=False)
    idxw = nc.declare_dram_parameter("idxw", [P, NCH * W], I16, isOutput=False)
    mq = nc.declare_dram_parameter("mq", [L], I32, isOutput=False)
    mk = nc.declare_dram_parameter("mk", [V], I32, isOutput=False)
    conf = nc.declare_dram_parameter("conf", [L, K], F32, isOutput=True)
    nconf = nc.declare_dram_parameter("nconf", [L], F32, isOutput=True)
    nidx = nc.declare_dram_parameter("nidx", [L], I32, isOutput=True)

    aug = nc.dram_tensor("aug_table", [V, C], F32).ap()

    with tile.TileContext(nc) as tc:
        with tc.tile_pool(name="prep", bufs=1) as prep:
            fkt = prep.tile([P, RT, C], F32)
            nc.sync.dma_start(out=fkt[:], in_=fk.rearrange("(r p) c -> p r c", p=P))
            mki = prep.tile([P, RT], I32)
            nc.sync.dma_start(out=mki[:], in_=mk.rearrange("(r p) -> p r", p=P))
            mkf = prep.tile([P, RT], F32)
            nc.vector.tensor_copy(mkf[:], mki[:])
            scale = prep.tile([P, RT], F32)
            nc.vector.tensor_scalar(
                scale[:], mkf[:], -TWO64, TWO64,
                mybir.AluOpType.mult, mybir.AluOpType.add)
            nc.vector.tensor_tensor(
                out=scale[:], in0=scale[:], in1=mkf[:], op=mybir.AluOpType.add)
            nc.vector.tensor_scalar_mul(scale[:], scale[:], 0.125)
            scale_b = bass.AP(tensor=scale[:].tensor, offset=scale[:].offset,
                              ap=[scale[:].ap[0], scale[:].ap[1], [0, C]])
            nc.vector.tensor_tensor(
                out=fkt[:], in0=fkt[:], in1=scale_b, op=mybir.AluOpType.mult)
            nc.sync.dma_start(out=aug.rearrange("(r p) c -> p r c", p=P), in_=fkt[:])

        with tc.tile_pool(name="res", bufs=1) as res, \
             tc.tile_pool(name="work", bufs=3) as work, \
             tc.tile_pool(name="iw", bufs=3) as iw, \
             tc.tile_pool(name="gath", bufs=3) as gath:

            fqt = res.tile([P, NCH, C], F32)
            nc.sync.dma_start(out=fqt[:], in_=fq.rearrange("(t p) c -> p t c", p=P))
            nc.vector.tensor_scalar_mul(fqt[:], fqt[:], 0.125)
            mqi = res.tile([P, NCH], I32)
            nc.sync.dma_start(out=mqi[:], in_=mq.rearrange("(t p) -> p t", p=P))
            mqf = res.tile([P, NCH], F32)
            nc.vector.tensor_copy(mqf[:], mqi[:])
            mqb = res.tile([P, NCH], F32)
            nc.vector.tensor_scalar(
                mqb[:], mqf[:], 1.0, 1.0e9,
                mybir.AluOpType.subtract, mybir.AluOpType.mult)
            negc = res.tile([P, 1], F32)
            nc.vector.memset(negc[:], NEG)
            nconf_acc = res.tile([P, NCH], F32)
            nidx_acc = res.tile([P, NCH], I32)

            qn = 0
            for t in range(NCH):
                it = iw.tile([P, W], I16)
                nc.sync.dma_start(out=it[:], in_=idxw[:, t * W:(t + 1) * W])
                g = gath.tile([P, K, C], F32)
                k0 = 0
                while k0 < K:
                    kn = min(GQ, K - k0)
                    ni = kn * P
                    nc.gpsimd.dma_gather(
                        g[:, k0:k0 + kn, :], aug[:, :],
                        it[:, k0 * GQ:k0 * GQ + ni // 16],
                        num_idxs=ni, num_idxs_reg=ni, elem_size=C,
                        queue_num=qn % NQUEUE)
                    qn += 1
                    k0 += kn
                nc.vector.tensor_tensor(
                    out=g[:], in0=g[:], in1=_bcast_mid(fqt[:, t, :], K),
                    op=mybir.AluOpType.mult)
                sims = work.tile([P, K], F32)
                nc.vector.tensor_reduce(
                    out=sims[:], in_=g[:], axis=mybir.AxisListType.X,
                    op=mybir.AluOpType.add)
                absb = work.tile([P, K], F32)
                nc.scalar.activation(absb[:], sims[:], mybir.ActivationFunctionType.Abs)
                mgt = work.tile([P, K], I32)
                nc.vector.tensor_scalar(
                    mgt[:], absb[:], THRESH, None, mybir.AluOpType.is_gt)
                nc.vector.copy_predicated(sims[:], mgt[:], _bcast_last(negc[:], K))
                nc.vector.tensor_scalar(
                    sims[:], sims[:], mqf[:, t:t + 1], mqb[:, t:t + 1],
                    mybir.AluOpType.mult, mybir.AluOpType.add)
                m8 = work.tile([P, 8], F32)
                nc.vector.max(m8[:], sims[:])
                ebias = work.tile([P, 1], F32)
                nc.vector.tensor_scalar_mul(ebias[:], m8[:, 0:1], -10.0)
                e = work.tile([P, K], F32)
                nc.scalar.activation(
                    e[:], sims[:], mybir.ActivationFunctionType.Exp,
                    bias=ebias[:], scale=10.0)
                z = work.tile([P, 1], F32)
                nc.vector.tensor_reduce(
                    out=z[:], in_=e[:], axis=mybir.AxisListType.X,
                    op=mybir.AluOpType.add)
                r = work.tile([P, 1], F32)
                nc.vector.reciprocal(r[:], z[:])
                conf_t = work.tile([P, K], F32)
                nc.vector.tensor_scalar_mul(conf_t[:], e[:], r[:])
                nc.sync.dma_start(out=conf[t * P:(t + 1) * P, :], in_=conf_t[:])
                c8 = work.tile([P, 8], F32)
                nc.vector.max(c8[:], conf_t[:])
                i8 = work.tile([P, 8], U32)
                nc.vector.max_index(i8[:], c8[:], conf_t[:])
                nc.vector.tensor_copy(nconf_acc[:, t:t + 1], c8[:, 0:1])
                nc.vector.tensor_copy(nidx_acc[:, t:t + 1], i8[:, 0:1].bitcast(I32))

            nc.sync.dma_start(out=nconf.rearrange("(t p) -> p t", p=P), in_=nconf_acc[:])
            nc.sync.dma_start(out=nidx.rearrange("(t p) -> p t", p=P), in_=nidx_acc[:])

    # Tile rotates Pool-DMA completion sems over 8 DMASW lanes; SWDGE locks
    # each lane to one queue (lane % nqueues). Make queue_num agree.
    import re
    for f in nc.m.functions:
        for bb in f.blocks:
            for inst in bb.instructions:
                if (type(inst).__name__ == "InstDMAGatherAnt"
                        and inst.sync_info and inst.sync_info.on_update):
                    for u in inst.sync_info.on_update:
                        m = re.match(r"DMASW(\d+)", u.ant_name or "")
                        if m:
                            inst.queue_num = int(m.group(1)) % NQUEUE
                            break
    return nc


def _make_idxw(idx_shard):
    """[L, K] i32 -> [128, (L/128)*800] i16: k-major flat (n = k*128+p),
    wrap-16 ([n%16, n//16]), replicated to all 8 Q7 core groups."""
    nch = idx_shard.shape[0] // P
    w = P * K // 16
    out = np.empty((P, nch * w), np.int16)
    for t in range(nch):
        blk = idx_shard[t * P:(t + 1) * P]
        wrap = blk.T.reshape(-1).reshape(-1, 16).T.astype(np.int16)
        out[:, t * w:(t + 1) * w] = np.tile(wrap, (8, 1))
    return out


_CACHED_NC = None


def _get_nc():
    global _CACHED_NC
    if _CACHED_NC is None:
        nc = bacc.Bacc("TRN2", target_bir_lowering=False, num_devices=8,
                       num_swdge_queues=NQUEUE)
        _build(nc)
        nc.compile()
        _CACHED_NC = nc
    return _CACHED_NC


def run_sharded(feat_c0, feat_c1, idx_c01, idx_c10, mask_c0, mask_c1,
                trace=False):
    """Returns (outputs_tuple, exec_time_ns_or_None)."""
    feat_c0 = np.ascontiguousarray(np.asarray(feat_c0, dtype=np.float32))
    feat_c1 = np.ascontiguousarray(np.asarray(feat_c1, dtype=np.float32))
    idx_c01 = np.ascontiguousarray(np.asarray(idx_c01, dtype=np.int32))
    idx_c10 = np.ascontiguousarray(np.asarray(idx_c10, dtype=np.int32))
    mask_c0 = np.ascontiguousarray(np.asarray(mask_c0, dtype=np.int32))
    mask_c1 = np.ascontiguousarray(np.asarray(mask_c1, dtype=np.int32))

    # shard c = b*4 + dir*2 + half
    in_maps = []
    shard_idx = []
    for b in range(2):
        for dr in range(2):
            for h in range(2):
                sl = slice(h * L, (h + 1) * L)
                if dr == 0:
                    fq, fk = feat_c0[b, sl], feat_c1[b]
                    ix, mqv, mkv = idx_c01[b, sl], mask_c0[b, sl], mask_c1[b]
                else:
                    fq, fk = feat_c1[b, sl], feat_c0[b]
                    ix, mqv, mkv = idx_c10[b, sl], mask_c1[b, sl], mask_c0[b]
                in_maps.append(dict(
                    fq=np.ascontiguousarray(fq), fk=np.ascontiguousarray(fk),
                    idxw=_make_idxw(ix), mq=np.ascontiguousarray(mqv),
                    mk=np.ascontiguousarray(mkv)))
                shard_idx.append(ix)

    nc = _get_nc()
    res = run_bass_kernel_spmd(nc, in_maps, list(range(8)), trace=trace)

    B, HW0 = 2, 2 * L
    conf01 = np.empty((B, HW0, K), np.float32)
    nconf01 = np.empty((B, HW0), np.float32)
    nidx01 = np.empty((B, HW0), np.int32)
    conf10 = np.empty((B, HW0, K), np.float32)
    nconf10 = np.empty((B, HW0), np.float32)
    nidx10 = np.empty((B, HW0), np.int32)
    for b in range(2):
        for dr in range(2):
            for h in range(2):
                ci = b * 4 + dr * 2 + h
                r = res.results[ci]
                sl = slice(h * L, (h + 1) * L)
                kstar = np.clip(r["nidx"], 0, K - 1).astype(np.int64)
                nid = np.take_along_axis(shard_idx[ci], kstar[:, None], 1)[:, 0]
                if dr == 0:
                    conf01[b, sl] = r["conf"]
                    nconf01[b, sl] = r["nconf"]
                    nidx01[b, sl] = nid
                else:
                    conf10[b, sl] = r["conf"]
                    nconf10[b, sl] = r["nconf"]
                    nidx10[b, sl] = nid
    outs = (conf01, nconf01, nidx01, conf10, nconf10, nidx10)
    return outs, res.exec_time_ns


def kernel(feat_c0, feat_c1, idx_c01, idx_c10, mask_c0, mask_c1):
    outs, _ = run_sharded(feat_c0, feat_c1, idx_c01, idx_c10, mask_c0, mask_c1)
    return outs


# revision 2
# speedup vs baseline: 1.0535x; 1.0535x over previous
"""CascadeMatching Trainium2 kernel (8-core SPMD, Bass/Tile).

Sharding: 8 cores = (2 batches x 2 directions x 2 query-halves). Each core:
queries [9600,64], full key table [19200,64], idx [9600,100].

Device algorithm per core (matches jax reference within fp rounding):
  aug[j] = 0.125*fk[j] * (mk[j] ? 1 : 2^64)        (masked keys -> huge rows)
  gather aug rows by idx via SWDGE dma_gather (4 queues, 1024 idx/inst)
  s' = <0.125*fq[l], gathered>                      (DVE mul + segmented reduce)
  sim = |s'| > 1e8 ? -1e9 : s'                      (magnitude mask detect)
  sim = sim*mq + (mq-1)*1e9                         (query mask)
  conf = softmax(10*sim); k* = first-occurrence argmax; nconf = max conf
Host: next_idx = idx[l, k*]; host also pre-wraps gather indices (int16,
k-major, wrap-16, group-replicated) as part of sharding.
"""

import numpy as np

import concourse.bass as bass
import concourse.bacc as bacc
import concourse.tile as tile
from concourse import mybir
from concourse.bass_utils import run_bass_kernel_spmd

F32 = mybir.dt.float32
I32 = mybir.dt.int32
I16 = mybir.dt.int16
U32 = mybir.dt.uint32

P = 128
C = 64
K = 100
L = 9600       # queries per core (half of HW0)
V = 19200
TWO64 = float(2.0**64)
THRESH = 1.0e8
NEG = -1.0e9
GQ = 8         # k-slots per gather instruction (1024 indices)
NQUEUE = 4


def _bcast_mid(ap, n_mid):
    return bass.AP(tensor=ap.tensor, offset=ap.offset,
                   ap=[ap.ap[0], [0, n_mid], ap.ap[1]])


def _bcast_last(ap, n):
    return bass.AP(tensor=ap.tensor, offset=ap.offset, ap=[ap.ap[0], [0, n]])


def _build(nc):
    NCH = L // P
    RT = V // P
    W = P * K // 16

    fq = nc.declare_dram_parameter("fq", [L, C], F32, isOutput=False)
    fk = nc.declare_dram_parameter("fk", [V, C], F32, isOutput=False)
    idxw = nc.declare_dram_parameter("idxw", [P, NCH * W], I16, isOutput=False)
    mq = nc.declare_dram_parameter("mq", [L], I32, isOutput=False)
    mk = nc.declare_dram_parameter("mk", [V], I32, isOutput=False)
    conf = nc.declare_dram_parameter("conf", [L, K], F32, isOutput=True)
    nconf = nc.declare_dram_parameter("nconf", [L], F32, isOutput=True)
    nidx = nc.declare_dram_parameter("nidx", [L], I32, isOutput=True)

    aug = nc.dram_tensor("aug_table", [V, C], F32).ap()

    with tile.TileContext(nc) as tc:
        with tc.tile_pool(name="prep", bufs=1) as prep:
            fkt = prep.tile([P, RT, C], F32)
            nc.sync.dma_start(out=fkt[:], in_=fk.rearrange("(r p) c -> p r c", p=P))
            mki = prep.tile([P, RT], I32)
            nc.sync.dma_start(out=mki[:], in_=mk.rearrange("(r p) -> p r", p=P))
            mkf = prep.tile([P, RT], F32)
            nc.vector.tensor_copy(mkf[:], mki[:])
            scale = prep.tile([P, RT], F32)
            nc.vector.tensor_scalar(
                scale[:], mkf[:], -TWO64, TWO64,
                mybir.AluOpType.mult, mybir.AluOpType.add)
            nc.vector.tensor_tensor(
                out=scale[:], in0=scale[:], in1=mkf[:], op=mybir.AluOpType.add)
            nc.vector.tensor_scalar_mul(scale[:], scale[:], 0.125)
            scale_b = bass.AP(tensor=scale[:].tensor, offset=scale[:].offset,
                              ap=[scale[:].ap[0], scale[:].ap[1], [0, C]])
            nc.vector.tensor_tensor(
                out=fkt[:], in0=fkt[:], in1=scale_b, op=mybir.AluOpType.mult)
            nc.sync.dma_start(out=aug.rearrange("(r p) c -> p r c", p=P), in_=fkt[:])

        with tc.tile_pool(name="res", bufs=1) as res, \
             tc.tile_pool(name="work", bufs=3) as work, \
             tc.tile_pool(name="iw", bufs=4) as iw, \
             tc.tile_pool(name="gath", bufs=4) as gath:

            fqt = res.tile([P, NCH, C], F32)
            nc.sync.dma_start(out=fqt[:], in_=fq.rearrange("(t p) c -> p t c", p=P))
            nc.vector.tensor_scalar_mul(fqt[:], fqt[:], 0.125)
            mqi = res.tile([P, NCH], I32)
            nc.sync.dma_start(out=mqi[:], in_=mq.rearrange("(t p) -> p t", p=P))
            mqf = res.tile([P, NCH], F32)
            nc.vector.tensor_copy(mqf[:], mqi[:])
            mqb = res.tile([P, NCH], F32)
            nc.vector.tensor_scalar(
                mqb[:], mqf[:], 1.0, 1.0e9,
                mybir.AluOpType.subtract, mybir.AluOpType.mult)
            negc = res.tile([P, 1], F32)
            nc.vector.memset(negc[:], NEG)
            nconf_acc = res.tile([P, NCH], F32)
            nidx_acc = res.tile([P, NCH], I32)

            qn = 0
            for t in range(NCH):
                it = iw.tile([P, W], I16)
                nc.sync.dma_start(out=it[:], in_=idxw[:, t * W:(t + 1) * W])
                g = gath.tile([P, K, C], F32)
                k0 = 0
                while k0 < K:
                    kn = min(GQ, K - k0)
                    ni = kn * P
                    nc.gpsimd.dma_gather(
                        g[:, k0:k0 + kn, :], aug[:, :],
                        it[:, k0 * GQ:k0 * GQ + ni // 16],
                        num_idxs=ni, num_idxs_reg=ni, elem_size=C,
                        queue_num=qn % NQUEUE)
                    qn += 1
                    k0 += kn
                nc.vector.tensor_tensor(
                    out=g[:], in0=g[:], in1=_bcast_mid(fqt[:, t, :], K),
                    op=mybir.AluOpType.mult)
                sims = work.tile([P, K], F32)
                nc.vector.tensor_reduce(
                    out=sims[:], in_=g[:], axis=mybir.AxisListType.X,
                    op=mybir.AluOpType.add)
                absb = work.tile([P, K], F32)
                nc.scalar.activation(absb[:], sims[:], mybir.ActivationFunctionType.Abs)
                mgt = work.tile([P, K], I32)
                nc.vector.tensor_scalar(
                    mgt[:], absb[:], THRESH, None, mybir.AluOpType.is_gt)
                nc.vector.copy_predicated(sims[:], mgt[:], _bcast_last(negc[:], K))
                nc.vector.tensor_scalar(
                    sims[:], sims[:], mqf[:, t:t + 1], mqb[:, t:t + 1],
                    mybir.AluOpType.mult, mybir.AluOpType.add)
                m8 = work.tile([P, 8], F32)
                nc.vector.max(m8[:], sims[:])
                ebias = work.tile([P, 1], F32)
                nc.vector.tensor_scalar_mul(ebias[:], m8[:, 0:1], -10.0)
                e = work.tile([P, K], F32)
                nc.scalar.activation(
                    e[:], sims[:], mybir.ActivationFunctionType.Exp,
                    bias=ebias[:], scale=10.0)
                z = work.tile([P, 1], F32)
                nc.vector.tensor_reduce(
                    out=z[:], in_=e[:], axis=mybir.AxisListType.X,
                    op=mybir.AluOpType.add)
                r = work.tile([P, 1], F32)
                nc.vector.reciprocal(r[:], z[:])
                conf_t = work.tile([P, K], F32)
                nc.vector.tensor_scalar_mul(conf_t[:], e[:], r[:])
                nc.sync.dma_start(out=conf[t * P:(t + 1) * P, :], in_=conf_t[:])
                c8 = work.tile([P, 8], F32)
                nc.vector.max(c8[:], conf_t[:])
                i8 = work.tile([P, 8], U32)
                nc.vector.max_index(i8[:], c8[:], conf_t[:])
                nc.vector.tensor_copy(nconf_acc[:, t:t + 1], c8[:, 0:1])
                nc.vector.tensor_copy(nidx_acc[:, t:t + 1], i8[:, 0:1].bitcast(I32))

            nc.sync.dma_start(out=nconf.rearrange("(t p) -> p t", p=P), in_=nconf_acc[:])
            nc.sync.dma_start(out=nidx.rearrange("(t p) -> p t", p=P), in_=nidx_acc[:])

    # Tile rotates Pool-DMA completion sems over 8 DMASW lanes; SWDGE locks
    # each lane to one queue (lane % nqueues). Make queue_num agree.
    import re
    for f in nc.m.functions:
        for bb in f.blocks:
            for inst in bb.instructions:
                if (type(inst).__name__ == "InstDMAGatherAnt"
                        and inst.sync_info and inst.sync_info.on_update):
                    for u in inst.sync_info.on_update:
                        m = re.match(r"DMASW(\d+)", u.ant_name or "")
                        if m:
                            inst.queue_num = int(m.group(1)) % NQUEUE
                            break
    return nc


def _make_idxw(idx_shard):
    """[L, K] i32 -> [128, (L/128)*800] i16: k-major flat (n = k*128+p),
    wrap-16 ([n%16, n//16]), replicated to all 8 Q7 core groups."""
    nch = idx_shard.shape[0] // P
    w = P * K // 16
    out = np.empty((P, nch * w), np.int16)
    for t in range(nch):
        blk = idx_shard[t * P:(t + 1) * P]
        wrap = blk.T.reshape(-1).reshape(-1, 16).T.astype(np.int16)
        out[:, t * w:(t + 1) * w] = np.tile(wrap, (8, 1))
    return out


_CACHED_NC = None


def _get_nc():
    global _CACHED_NC
    if _CACHED_NC is None:
        nc = bacc.Bacc("TRN2", target_bir_lowering=False, num_devices=8,
                       num_swdge_queues=NQUEUE)
        _build(nc)
        nc.compile()
        _CACHED_NC = nc
    return _CACHED_NC


def run_sharded(feat_c0, feat_c1, idx_c01, idx_c10, mask_c0, mask_c1,
                trace=False):
    """Returns (outputs_tuple, exec_time_ns_or_None)."""
    feat_c0 = np.ascontiguousarray(np.asarray(feat_c0, dtype=np.float32))
    feat_c1 = np.ascontiguousarray(np.asarray(feat_c1, dtype=np.float32))
    idx_c01 = np.ascontiguousarray(np.asarray(idx_c01, dtype=np.int32))
    idx_c10 = np.ascontiguousarray(np.asarray(idx_c10, dtype=np.int32))
    mask_c0 = np.ascontiguousarray(np.asarray(mask_c0, dtype=np.int32))
    mask_c1 = np.ascontiguousarray(np.asarray(mask_c1, dtype=np.int32))

    # shard c = b*4 + dir*2 + half
    in_maps = []
    shard_idx = []
    for b in range(2):
        for dr in range(2):
            for h in range(2):
                sl = slice(h * L, (h + 1) * L)
                if dr == 0:
                    fq, fk = feat_c0[b, sl], feat_c1[b]
                    ix, mqv, mkv = idx_c01[b, sl], mask_c0[b, sl], mask_c1[b]
                else:
                    fq, fk = feat_c1[b, sl], feat_c0[b]
                    ix, mqv, mkv = idx_c10[b, sl], mask_c1[b, sl], mask_c0[b]
                in_maps.append(dict(
                    fq=np.ascontiguousarray(fq), fk=np.ascontiguousarray(fk),
                    idxw=_make_idxw(ix), mq=np.ascontiguousarray(mqv),
                    mk=np.ascontiguousarray(mkv)))
                shard_idx.append(ix)

    nc = _get_nc()
    res = run_bass_kernel_spmd(nc, in_maps, list(range(8)), trace=trace)

    B, HW0 = 2, 2 * L
    conf01 = np.empty((B, HW0, K), np.float32)
    nconf01 = np.empty((B, HW0), np.float32)
    nidx01 = np.empty((B, HW0), np.int32)
    conf10 = np.empty((B, HW0, K), np.float32)
    nconf10 = np.empty((B, HW0), np.float32)
    nidx10 = np.empty((B, HW0), np.int32)
    for b in range(2):
        for dr in range(2):
            for h in range(2):
                ci = b * 4 + dr * 2 + h
                r = res.results[ci]
                sl = slice(h * L, (h + 1) * L)
                kstar = np.clip(r["nidx"], 0, K - 1).astype(np.int64)
                nid = np.take_along_axis(shard_idx[ci], kstar[:, None], 1)[:, 0]
                if dr == 0:
                    conf01[b, sl] = r["conf"]
                    nconf01[b, sl] = r["nconf"]
                    nidx01[b, sl] = nid
                else:
                    conf10[b, sl] = r["conf"]
                    nconf10[b, sl] = r["nconf"]
                    nidx10[b, sl] = nid
    outs = (conf01, nconf01, nidx01, conf10, nconf10, nidx10)
    return outs, res.exec_time_ns


def kernel(feat_c0, feat_c1, idx_c01, idx_c10, mask_c0, mask_c1):
    outs, _ = run_sharded(feat_c0, feat_c1, idx_c01, idx_c10, mask_c0, mask_c1)
    return outs
